# revision 4
# baseline (speedup 1.0000x reference)
"""Trainium2 Bass kernel for DGL HyperGCNII conv (hypergraph message passing).

Computation (reference):
    Xe = segment_sum(X[g1_src], g1_dst, E) * degE          # nodes -> hyperedges
    Xv = segment_sum(Xe[g2_src], g2_dst, N) * degV         # hyperedges -> nodes
    Xi = (1-a)*Xv + a*X0
    out = (1-b)*Xi + b*(Xi @ W.T)

Strategy (8 NeuronCores, vertex-cut graph parallelism):
- Shard nodes across cores. Phase 1: core c processes nnz whose g1_src lives in
  its shard; gathers rows with SWDGE dma_gather (fp16) and segment-sums them
  into per-edge-block PSUM accumulators using one-hot selection matmuls
  (S[t,j] = (seg[t]==j) * degE[dst[t]], built on DVE from an iota compare with
  degE folded in).  Partial Xe (all edges) per core.
- AllReduce (fp16) the Xe partials across the 8 cores.
- Phase 2: core c processes nnz whose g2_dst is in its shard; gathers Xe rows,
  segment-sums transposed (stationary=G) so the result lands as Xv^T[d,v] with
  degV*(1-a) folded into S; adds a*X0^T, multiplies by M = (1-b)I + b*W via a
  second matmul which also un-transposes, writes out rows.

All indices / segment slots / fold weights are precomputed host-side as int16 /
f32 metadata (index-only preprocessing); data math happens on device.
"""

import os
import numpy as np
from contextlib import ExitStack
from dataclasses import dataclass

import concourse.bass as bass
import concourse.tile as tile
from concourse import bacc, mybir
from concourse.bass_utils import run_bass_kernel_spmd
from concourse.library_config import mlp

P = 128
F32 = mybir.dt.float32
F16 = mybir.dt.float16
I16 = mybir.dt.int16


@dataclass(frozen=True)
class Cfg:
    n_nodes: int = 100000
    n_edges: int = 20000
    d: int = 128
    ncores: int = 8
    b1: int = 4      # edge blocks per dma_gather call (phase 1)
    b2: int = 4      # node blocks per dma_gather call (phase 2)

    @property
    def nb_v(self):  # node blocks per core
        per_core = -(-self.n_nodes // (self.ncores * P))
        return per_core

    @property
    def nsh(self):   # nodes per core (padded)
        return self.nb_v * P

    @property
    def n_pad(self):
        return self.nsh * self.ncores

    @property
    def nb_e(self):  # edge blocks (global, replicated on each core)
        return -(-self.n_edges // P)

    @property
    def e_pad(self):
        return self.nb_e * P


CFG = Cfg()


def _groups(nblocks, bsz):
    return [(b0, min(b0 + bsz, nblocks)) for b0 in range(0, nblocks, bsz)]


def _build_phase_meta(src_idx, dst_local, deg_w, nblocks, K):
    """Pad/sort one phase's nnz into fixed [nblocks, K*128] slot arrays."""
    order = np.argsort(dst_local, kind="stable")
    s = src_idx[order].astype(np.int64)
    dl = dst_local[order].astype(np.int64)
    w = deg_w[order].astype(np.float32)
    blk = dl // P
    counts = np.bincount(blk, minlength=nblocks)
    off = np.zeros(nblocks + 1, np.int64)
    np.cumsum(counts, out=off[1:])
    pos = np.arange(len(dl)) - off[blk]
    tgt = blk * (K * P) + pos
    idx = np.zeros(nblocks * K * P, np.int16)
    seg = np.full(nblocks * K * P, 999.0, np.float32)
    dw = np.zeros(nblocks * K * P, np.float32)
    idx[tgt] = s.astype(np.int16)
    seg[tgt] = (dl - blk * P).astype(np.float32)
    dw[tgt] = w
    return (idx.reshape(nblocks, K * P), seg.reshape(nblocks, K * P),
            dw.reshape(nblocks, K * P))


def _pack_idx_calls(idx, groups, K):
    """Pack gather indices into the SWDGE 16-wrap layout, one segment per call."""
    cols = []
    for b0, b1 in groups:
        flat = idx[b0:b1].reshape(-1)
        wrap = flat.reshape(-1, 16).T          # [16, L/16]
        cols.append(np.tile(wrap, (8, 1)))     # [128, L/16]
    return np.ascontiguousarray(np.concatenate(cols, axis=1))


def _seg_cols(arr, nblocks, K):
    """[nblocks, K*128] -> [128, nblocks*K]; tile t's slot values in col t."""
    return np.ascontiguousarray(arr.reshape(nblocks * K, P).T)


_PROGRAM_CACHE = {}


def build_program(K1, K2, alpha, cfg=CFG, compile=True):
    key = (K1, K2, float(alpha), cfg)
    if key in _PROGRAM_CACHE:
        return _PROGRAM_CACHE[key]

    D = cfg.d
    NSH, NB_V, NB_E, E_PAD = cfg.nsh, cfg.nb_v, cfg.nb_e, cfg.e_pad

    nc = bacc.Bacc("TRN2", target_bir_lowering=False, debug=False,
                   num_devices=cfg.ncores)

    xsh = nc.dram_tensor("xsh", [NSH, D], F32, kind="ExternalInput")
    x0t = nc.dram_tensor("x0t", [D, NSH], F32, kind="ExternalInput")
    idx1 = nc.dram_tensor("idx1", [P, NB_E * K1 * 8], I16, kind="ExternalInput")
    seg1 = nc.dram_tensor("seg1", [P, NB_E * K1], F32, kind="ExternalInput")
    dw1 = nc.dram_tensor("dw1", [P, NB_E * K1], F32, kind="ExternalInput")
    idx2 = nc.dram_tensor("idx2", [P, NB_V * K2 * 8], I16, kind="ExternalInput")
    seg2 = nc.dram_tensor("seg2", [P, NB_V * K2], F32, kind="ExternalInput")
    dw2 = nc.dram_tensor("dw2", [P, NB_V * K2], F32, kind="ExternalInput")
    m_arr = nc.dram_tensor("m_arr", [D, D], F16, kind="ExternalInput")
    out = nc.dram_tensor("out", [NSH, D], F32, kind="ExternalOutput")

    g1 = _groups(NB_E, cfg.b1)
    g2 = _groups(NB_V, cfg.b2)

    with tile.TileContext(nc) as tc, ExitStack() as ctx:
        nc.gpsimd.load_library(mlp)
        const = ctx.enter_context(tc.tile_pool(name="const", bufs=1))
        idxp = ctx.enter_context(tc.tile_pool(name="idxp", bufs=1))
        xp = ctx.enter_context(tc.tile_pool(name="xp", bufs=1))
        xcp = ctx.enter_context(tc.tile_pool(name="xcp", bufs=2))
        gp = ctx.enter_context(tc.tile_pool(name="gp", bufs=2))
        sp = ctx.enter_context(tc.tile_pool(name="sp", bufs=4))
        ep = ctx.enter_context(tc.tile_pool(name="ep", bufs=2))
        ps_acc = ctx.enter_context(tc.tile_pool(name="psacc", bufs=4, space="PSUM"))
        ps_mm = ctx.enter_context(tc.tile_pool(name="psmm", bufs=2, space="PSUM"))
        dram = ctx.enter_context(tc.tile_pool(name="dram", bufs=1, space="DRAM"))

        iota_t = const.tile([P, P], F32)
        nc.gpsimd.iota(iota_t[:], pattern=[[1, P]], base=0, channel_multiplier=0,
                       allow_small_or_imprecise_dtypes=True)
        m_t = const.tile([D, D], F16)
        nc.sync.dma_start(m_t[:], m_arr[:, :])

        idx1_t = idxp.tile([P, NB_E * K1 * 8], I16)
        seg1_t = idxp.tile([P, NB_E * K1], F32)
        dw1_t = idxp.tile([P, NB_E * K1], F32)
        idx2_t = idxp.tile([P, NB_V * K2 * 8], I16)
        seg2_t = idxp.tile([P, NB_V * K2], F32)
        dw2_t = idxp.tile([P, NB_V * K2], F32)
        nc.sync.dma_start(idx1_t[:], idx1[:, :])
        nc.sync.dma_start(seg1_t[:], seg1[:, :])
        nc.sync.dma_start(dw1_t[:], dw1[:, :])
        nc.sync.dma_start(idx2_t[:], idx2[:, :])
        nc.sync.dma_start(seg2_t[:], seg2[:, :])
        nc.sync.dma_start(dw2_t[:], dw2[:, :])

        # ---- cast X shard f32 -> f16 into DRAM (gather table) ----
        xsh16 = dram.tile([NSH, D], F16)
        xsh_flat = xsh.ap().rearrange("(p r) d -> p (r d)", p=P)
        xsh16_flat = xsh16[:].rearrange("(p r) d -> p (r d)", p=P)
        CH = 4
        chw = (NSH // P) * D // CH
        for cidx in range(CH):
            xin = xcp.tile([P, chw], F32, tag="xin")
            nc.sync.dma_start(xin[:], xsh_flat[:, cidx * chw:(cidx + 1) * chw])
            x16 = xcp.tile([P, chw], F16, tag="x16")
            nc.vector.tensor_copy(x16[:], xin[:])
            nc.sync.dma_start(xsh16_flat[:, cidx * chw:(cidx + 1) * chw], x16[:])

        # ---- phase 1: nodes -> hyperedges (partial Xe, deg-folded) ----
        xe_part = dram.tile([E_PAD, D], F16)
        xe_full = dram.tile([E_PAD, D], F16)
        off16 = 0
        for (b0, b1) in g1:
            nb = b1 - b0
            L = nb * K1 * P
            g_t = gp.tile([P, nb * K1, P], F16, tag="g1")
            nc.gpsimd.dma_gather(g_t[:], xsh16[:, :],
                                 idx1_t[:, off16:off16 + L // 16], L, L, D,
                                 single_packet=False)
            off16 += L // 16
            xe_o = ep.tile([P, nb, P], F16, tag="xeo")
            for b in range(b0, b1):
                acc = ps_acc.tile([P, P], F32, tag="acc", space="PSUM")
                for k in range(K1):
                    tg = b * K1 + k
                    tl = (b - b0) * K1 + k
                    s_t = sp.tile([P, P], F16, tag="s")
                    nc.vector.tensor_scalar(
                        out=s_t[:], in0=iota_t[:],
                        scalar1=seg1_t[:, tg:tg + 1], scalar2=dw1_t[:, tg:tg + 1],
                        op0=mybir.AluOpType.is_equal, op1=mybir.AluOpType.mult)
                    nc.tensor.matmul(acc[:], lhsT=s_t[:], rhs=g_t[:, tl, :],
                                     start=(k == 0), stop=(k == K1 - 1))
                nc.vector.tensor_copy(xe_o[:, b - b0, :], acc[:])
            dst = xe_part[b0 * P:b1 * P, :].rearrange("(b j) d -> j b d", j=P)
            nc.sync.dma_start(dst, xe_o[:])

        # ---- AllReduce Xe partials across cores ----
        if os.environ.get("K_SKIP_CC"):
            nc.gpsimd.dma_start(xe_full[:], xe_part[:])
        else:
            nc.gpsimd.collective_compute(
                "AllReduce", mybir.AluOpType.add,
                replica_groups=[list(range(cfg.ncores))],
                ins=[xe_part[:].opt()], outs=[xe_full[:].opt()])

        # ---- phase 2: hyperedges -> nodes (transposed acc), epilogue ----
        x0_t = xp.tile([D, NSH], F16, tag="x0")
        nc.gpsimd.dma_start(x0_t[:], x0t[:, :])  # SWDGE cast f32->f16
        a_const = float(alpha)
        nc.vector.tensor_scalar(out=x0_t[:], in0=x0_t[:], scalar1=a_const,
                                scalar2=None, op0=mybir.AluOpType.mult)

        off16 = 0
        for (b0, b1) in g2:
            nb = b1 - b0
            L = nb * K2 * P
            g_t = gp.tile([P, nb * K2, P], F16, tag="g2")
            nc.gpsimd.dma_gather(g_t[:], xe_full[:, :],
                                 idx2_t[:, off16:off16 + L // 16], L, L, D,
                                 single_packet=False)
            off16 += L // 16
            out_o = ep.tile([P, nb, P], F32, tag="outo")
            for b in range(b0, b1):
                acc = ps_acc.tile([P, P], F32, tag="acc", space="PSUM")
                for k in range(K2):
                    tg = b * K2 + k
                    tl = (b - b0) * K2 + k
                    s_t = sp.tile([P, P], F16, tag="s")
                    nc.vector.tensor_scalar(
                        out=s_t[:], in0=iota_t[:],
                        scalar1=seg2_t[:, tg:tg + 1], scalar2=dw2_t[:, tg:tg + 1],
                        op0=mybir.AluOpType.is_equal, op1=mybir.AluOpType.mult)
                    nc.tensor.matmul(acc[:], lhsT=g_t[:, tl, :], rhs=s_t[:],
                                     start=(k == 0), stop=(k == K2 - 1))
                xiT = ep.tile([P, P], F16, tag="xiT")
                nc.vector.tensor_tensor(out=xiT[:], in0=acc[:],
                                        in1=x0_t[:, b * P:(b + 1) * P],
                                        op=mybir.AluOpType.add)
                mm = ps_mm.tile([P, P], F32, tag="mm", space="PSUM")
                nc.tensor.matmul(mm[:], lhsT=xiT[:], rhs=m_t[:],
                                 start=True, stop=True)
                nc.vector.tensor_copy(out_o[:, b - b0, :], mm[:])
            dst = out.ap()[b0 * P:b1 * P, :].rearrange("(b j) d -> j b d", j=P)
            nc.sync.dma_start(dst, out_o[:])

    if compile:
        nc.compile()
    _PROGRAM_CACHE[key] = nc
    return nc


def build_in_maps(inputs, cfg=CFG):
    """Host-side sharding + index preprocessing. Returns (in_maps, K1, K2, alpha)."""
    D = cfg.d
    NSH, NB_V, NB_E = cfg.nsh, cfg.nb_v, cfg.nb_e

    X = np.asarray(inputs["X"], np.float32)
    X0 = np.asarray(inputs["X0"], np.float32)
    degE = np.asarray(inputs["degE"], np.float32).reshape(-1)
    degV = np.asarray(inputs["degV"], np.float32).reshape(-1)
    alpha = float(np.asarray(inputs["alpha"]).reshape(-1)[0])
    beta = float(np.asarray(inputs["beta"]).reshape(-1)[0])
    W = np.asarray(inputs["W_w"], np.float32)
    g1_src = np.asarray(inputs["g1_src"]).astype(np.int64)
    g1_dst = np.asarray(inputs["g1_dst"]).astype(np.int64)
    g2_src = np.asarray(inputs["g2_src"]).astype(np.int64)
    g2_dst = np.asarray(inputs["g2_dst"]).astype(np.int64)

    per_core = []
    K1_req, K2_req = 0, 0
    for c in range(cfg.ncores):
        lo, hi = c * NSH, (c + 1) * NSH
        m1 = (g1_src >= lo) & (g1_src < hi)
        m2 = (g2_dst >= lo) & (g2_dst < hi)
        s1, d1 = g1_src[m1] - lo, g1_dst[m1]
        s2, d2 = g2_src[m2], g2_dst[m2] - lo
        per_core.append((s1, d1, s2, d2))
        if len(d1):
            K1_req = max(K1_req, int(np.bincount(d1 // P, minlength=NB_E).max()))
        if len(d2):
            K2_req = max(K2_req, int(np.bincount(d2 // P, minlength=NB_V).max()))
    K1 = max(1, -(-K1_req // P))
    K2 = max(1, -(-K2_req // P))

    g1g = _groups(NB_E, cfg.b1)
    g2g = _groups(NB_V, cfg.b2)

    M = (1.0 - beta) * np.eye(D, dtype=np.float32) + beta * W
    m_arr = np.ascontiguousarray(M.T).astype(np.float16)  # [d, j] = M[j, d]

    X_pad = np.zeros((cfg.n_pad, D), np.float32)
    X_pad[:cfg.n_nodes] = X
    X0_pad = np.zeros((cfg.n_pad, D), np.float32)
    X0_pad[:cfg.n_nodes] = X0

    in_maps = []
    for c in range(cfg.ncores):
        s1, d1, s2, d2 = per_core[c]
        i1, sg1, w1 = _build_phase_meta(s1, d1, degE[d1], NB_E, K1)
        i2, sg2, w2 = _build_phase_meta(s2, d2, degV[d2 + c * NSH] * (1.0 - alpha),
                                        NB_V, K2)
        in_maps.append({
            "xsh": np.ascontiguousarray(X_pad[c * NSH:(c + 1) * NSH]),
            "x0t": np.ascontiguousarray(X0_pad[c * NSH:(c + 1) * NSH].T),
            "idx1": _pack_idx_calls(i1, g1g, K1),
            "seg1": _seg_cols(sg1, NB_E, K1),
            "dw1": _seg_cols(w1, NB_E, K1),
            "idx2": _pack_idx_calls(i2, g2g, K2),
            "seg2": _seg_cols(sg2, NB_V, K2),
            "dw2": _seg_cols(w2, NB_V, K2),
            "m_arr": m_arr,
        })
    return in_maps, K1, K2, alpha


def _enable_axon_trace_hook():
    """Best-effort: register the NTFF profile hook so BASS_TRACE=1 works."""
    try:
        import sys, types
        import antenv  # noqa: F401
        if "antenv.axon_hooks" not in sys.modules:
            from trn_agent_boot.trn_boot import _ntff_profile_via_ctypes
            hook = _ntff_profile_via_ctypes("/opt/axon/libaxon_pjrt.so")
            hm = types.ModuleType("antenv.axon_hooks")
            hm.get_axon_ntff_profile_hook = lambda: hook
            hm.set_axon_ntff_profile_hook = lambda h: None
            sys.modules["antenv.axon_hooks"] = hm
        import concourse.bass_utils as bu
        bu.upload_artifacts = lambda tmpdir: "local://" + tmpdir
    except Exception:
        pass


LAST_EXEC_TIME_NS = None


def kernel(**inputs):
    global LAST_EXEC_TIME_NS
    cfg = CFG
    in_maps, K1, K2, alpha = build_in_maps(inputs, cfg)

    if os.environ.get("BASS_TRACE"):
        _enable_axon_trace_hook()

    nc = build_program(K1, K2, alpha, cfg)
    res = run_bass_kernel_spmd(nc, in_maps, core_ids=list(range(cfg.ncores)))
    LAST_EXEC_TIME_NS = res.exec_time_ns

    out = np.concatenate([res.results[c]["out"] for c in range(cfg.ncores)], axis=0)
    return np.ascontiguousarray(out[:cfg.n_nodes]).astype(np.float32)
